# revision 11
# baseline (speedup 1.0000x reference)
"""Bahdanau-style attention on 8 Trainium2 NeuronCores.

Per batch row b (shard: 8 batch rows per core, weights replicated):
    proj   = enc[b] @ W_enc + b_enc            # (T, A)
    energy = tanh(proj + dec[b] @ W_dec)       # (T, A)
    scores = energy @ v_att                    # (T,)  (+b_att cancels in softmax)
    w      = softmax(scores)
    out    = (w @ enc[b], w)

Core dataflow per batch:
  1. SWDGE cast-DMA loads enc fp32 -> bf16 native (t-parts, e-free); the
     dtype cast is free (runs at full HBM rate).
  2. encT (e-parts, t-free) built two ways in parallel (they use different
     resources): XBAR dma-transpose for the first XBAR_TT t-blocks (shares
     SDMA capacity with the loads), PE matmul-vs-identity transposes +
     DVE/ACT psum drains for the rest.
  3. projT chunks = W.T @ encT on PE (W stationary, fp32 accum), tanh+bias
     applied by ACT straight out of PSUM (bias = dec_projT + b_enc,
     per-partition AP), energy stored bf16 (a-parts, t-free).
  4. scores = v.T @ energy on PE (v stationary, streams energy, N=512
     chunks) -> (1, T) psum rows; softmax on the single lane: ACT exp with
     free-dim accumulate, DVE reciprocal + scale. No max-subtraction:
     |scores| <= sum|v| < 16, exp is safe in fp32.
  5. w row (bf16) roundtrips DRAM and comes back XBAR-transposed as
     (128 t-parts, 32) for the context matmul.
  6. context = w.T @ enc_bf on PE (w column stationary, streams native enc).
"""

import os
import sys

import numpy as np

for _p in ("/opt/trn_rl_repo",):
    if _p not in sys.path:
        sys.path.insert(0, _p)

B, T, E, A = 64, 4096, 512, 256
N_CORES = 8
BPC = B // N_CORES          # batches per core
TT = T // 128               # 32 t-blocks
EJ = E // 128               # 4 e-blocks
AM = A // 128               # 2 a-halves
TC = 512                    # t-chunk (psum bank)
NTC = T // TC               # 8 chunks
XBAR_TT = 12                # t-blocks transposed via XBAR DMA (rest: PE)

_CACHE = {}


def _build():
    import concourse.mybir as mybir
    import concourse.tile as tile
    from concourse import bacc
    from concourse.masks import make_identity

    fp32 = mybir.dt.float32
    bf16 = mybir.dt.bfloat16

    nc = bacc.Bacc("TRN2", target_bir_lowering=False, num_devices=N_CORES)

    enc = nc.dram_tensor("enc", [BPC, T, E], fp32, kind="ExternalInput").ap()
    dec = nc.dram_tensor("dec", [BPC, E], fp32, kind="ExternalInput").ap()
    w_enc = nc.dram_tensor("w_enc", [E, A], fp32, kind="ExternalInput").ap()
    b_enc = nc.dram_tensor("b_enc", [A], fp32, kind="ExternalInput").ap()
    w_dec = nc.dram_tensor("w_dec", [E, A], fp32, kind="ExternalInput").ap()
    v_att = nc.dram_tensor("v_att", [A], fp32, kind="ExternalInput").ap()
    ctx_out = nc.dram_tensor("ctx_out", [BPC, E], fp32, kind="ExternalOutput").ap()
    attn_out = nc.dram_tensor("attn_out", [BPC, T], fp32, kind="ExternalOutput").ap()

    with tile.TileContext(nc) as tc:
        with (
            tc.tile_pool(name="const", bufs=1) as cpool,
            tc.tile_pool(name="enc", bufs=3) as enc_pool,
            tc.tile_pool(name="encT", bufs=1) as encT_pool,
            tc.tile_pool(name="energy", bufs=2) as en_pool,
            tc.tile_pool(name="small", bufs=2) as spool,
            tc.tile_pool(name="psum_proj", bufs=2, space="PSUM") as pp,
            tc.tile_pool(name="psum_tr", bufs=2, space="PSUM") as pt,
            tc.tile_pool(name="psum_sc", bufs=2, space="PSUM") as psc,
            tc.tile_pool(name="psum_ctx", bufs=1, space="PSUM") as pctx,
            tc.tile_pool(name="psum_misc", bufs=1, space="PSUM") as pmisc,
        ):
            # ---- constants / one-time prep -------------------------------
            wenc_bf = cpool.tile([128, EJ, A], bf16)        # [e_in, e_blk, a]
            nc.gpsimd.dma_start(
                wenc_bf[:], w_enc.rearrange("(j p) a -> p j a", p=128)
            )
            wdec_bf = cpool.tile([128, EJ, A], bf16)
            nc.gpsimd.dma_start(
                wdec_bf[:], w_dec.rearrange("(j p) a -> p j a", p=128)
            )
            v_sb = cpool.tile([128, AM], bf16)              # v[m*128+p]
            b_sb = cpool.tile([128, AM], fp32)
            for m in range(AM):
                nc.gpsimd.dma_start(v_sb[:, m : m + 1], v_att[m * 128 : (m + 1) * 128, None])
                nc.sync.dma_start(b_sb[:, m : m + 1], b_enc[m * 128 : (m + 1) * 128, None])

            ident_bf = cpool.tile([128, 128], bf16)
            make_identity(nc, ident_bf[:])
            ident_f = cpool.tile([128, 128], fp32)
            make_identity(nc, ident_f[:])
            ones_col = cpool.tile([128, 1], fp32)
            nc.vector.memset(ones_col[:], 1.0)
            ones_row = cpool.tile([1, 128], fp32)
            nc.vector.memset(ones_row[:], 1.0)

            # decoder projection -> per-batch per-partition bias
            dec_bf = cpool.tile([8, E], bf16)
            nc.gpsimd.dma_start(dec_bf[:], dec[:, :])
            decT_bf = cpool.tile([128, EJ, BPC], bf16)      # dec.T blocks
            for j in range(EJ):
                ps_t = pmisc.tile([128, BPC], bf16, tag="misc")
                nc.tensor.transpose(
                    ps_t[:], dec_bf[:, j * 128 : (j + 1) * 128], ident_bf[:8, :8]
                )
                nc.vector.tensor_copy(decT_bf[:, j, :], ps_t[:])
            biasT = cpool.tile([128, AM, BPC], fp32)        # dprojT + b_enc
            for m in range(AM):
                ps_dp = pmisc.tile([128, BPC], fp32, tag="misc")
                for j in range(EJ):
                    nc.tensor.matmul(
                        ps_dp[:],
                        lhsT=wdec_bf[:, j, m * 128 : (m + 1) * 128],
                        rhs=decT_bf[:, j, :],
                        start=(j == 0),
                        stop=(j == EJ - 1),
                    )
                nc.vector.tensor_tensor(
                    biasT[:, m, :],
                    ps_dp[:],
                    b_sb[:, m : m + 1].to_broadcast((128, BPC)),
                    mybir.AluOpType.add,
                )

            # ---- main per-batch pipeline ---------------------------------
            for b in range(BPC):
                # 1. cast-load native: enc_bf[p, tt, e] = enc[tt*128+p, e]
                enc_bf = enc_pool.tile([128, TT, E], bf16, tag="enc_bf")
                enc_r = enc[b].rearrange("(tt p) e -> p tt e", p=128)
                if b == 0:  # fine chunks so PE transposes start early
                    for lc in range(0, TT, 4):
                        nc.gpsimd.dma_start(
                            enc_bf[:, lc : lc + 4, :], enc_r[:, lc : lc + 4, :]
                        )
                else:
                    nc.gpsimd.dma_start(enc_bf[:], enc_r)
                # 2. encT2[p, tt, j*128+ti] = enc[tt*128+ti, j*128+p]
                encT2 = encT_pool.tile([128, TT, EJ * 128], bf16, tag="encT")
                # 2a. XBAR for the last XBAR_TT t-blocks, groups of 4
                for tt0 in range(TT - XBAR_TT, TT, 4):
                    nc.sync.dma_start_transpose(
                        encT2[:, tt0 : tt0 + 4, :].rearrange(
                            "p tt (j ti) -> p (tt j) ti", ti=128
                        ),
                        enc_bf[:, tt0 : tt0 + 4, :].rearrange("p tt e -> p (tt e)"),
                    )
                # 2b. PE transposes for the first blocks; drains on DVE
                for tt in range(0, TT - XBAR_TT):
                    ps_tr = pt.tile([128, 512], fp32, tag="tr")
                    for k in range(EJ):
                        nc.tensor.matmul(
                            ps_tr[:, k * 128 : (k + 1) * 128],
                            lhsT=enc_bf[:, tt, k * 128 : (k + 1) * 128],
                            rhs=ident_bf[:],
                        )
                    nc.vector.tensor_copy(encT2[:, tt, :], ps_tr[:])
                # 3. projection chunks + tanh (tcx outer so scores can follow)
                energy = en_pool.tile([128, AM, T], bf16, tag="energy")
                ps_s_list = []
                for tcx in range(NTC):
                    for m in range(AM):
                        ps = pp.tile([128, TC], fp32, tag="proj")
                        for j in range(EJ):
                            nc.tensor.matmul(
                                ps[:],
                                lhsT=wenc_bf[:, j, m * 128 : (m + 1) * 128],
                                rhs=encT2[:, 4 * tcx : 4 * tcx + 4, j * 128 : (j + 1) * 128],
                                start=(j == 0),
                                stop=(j == EJ - 1),
                            )
                        nc.scalar.activation(
                            energy[:, m, tcx * TC : (tcx + 1) * TC],
                            ps[:],
                            mybir.ActivationFunctionType.Tanh,
                            bias=biasT[:, m, b : b + 1],
                            scale=1.0,
                        )
                # 4. scores in (128 t-parts, 32) via energy-block-stationary
                ps_sc = psc.tile([128, TT], fp32, tag="scores")
                for tt in range(TT):
                    for m in range(AM):
                        nc.tensor.matmul(
                            ps_sc[:, tt : tt + 1],
                            lhsT=energy[:, m, tt * 128 : (tt + 1) * 128],
                            rhs=v_sb[:, m : m + 1],
                            start=(m == 0),
                            stop=(m == AM - 1),
                        )
                # 5. softmax across all 128x32 entries
                expw = spool.tile([128, TT], fp32, tag="expw")
                sumrow = spool.tile([128, 1], fp32, tag="sumrow")
                nc.scalar.activation(
                    expw[:],
                    ps_sc[:],
                    mybir.ActivationFunctionType.Exp,
                    accum_out=sumrow[:],
                )
                ps_tot = pmisc.tile([1, 1], fp32, tag="misc")
                nc.tensor.matmul(ps_tot[:], lhsT=ones_col[:], rhs=sumrow[:])
                inv = spool.tile([1, 1], fp32, tag="inv")
                nc.vector.reciprocal(inv[:], ps_tot[:])
                ps_invb = pmisc.tile([128, 1], fp32, tag="misc")
                nc.tensor.matmul(ps_invb[:], lhsT=ones_row[:], rhs=inv[:])
                w_f = spool.tile([128, TT], fp32, tag="w_f")
                nc.vector.tensor_tensor(
                    w_f[:],
                    expw[:],
                    ps_invb[:].to_broadcast((128, TT)),
                    mybir.AluOpType.mult,
                )
                w_bf = spool.tile([128, TT], bf16, tag="w_bf")
                nc.vector.tensor_copy(w_bf[:], w_f[:])
                # 6. context
                ps_cx = pctx.tile([1, E], fp32, tag="ctx")
                for tt in range(TT):
                    nc.tensor.matmul(
                        ps_cx[:],
                        lhsT=w_bf[:, tt : tt + 1],
                        rhs=enc_bf[:, tt, :],
                        start=(tt == 0),
                        stop=(tt == TT - 1),
                    )
                cx_sb = spool.tile([1, E], fp32, tag="cx_sb")
                nc.vector.tensor_copy(cx_sb[:], ps_cx[:])
                nc.sync.dma_start(ctx_out[b, None, :], cx_sb[:])
                # 7. weights out: transpose (128, 32) -> (32, 128) rows
                ps_wT = pmisc.tile([32, 128], fp32, tag="misc")
                nc.tensor.transpose(ps_wT[:], w_f[:], ident_f[:])
                wT_sb = spool.tile([32, 128], fp32, tag="wT_sb")
                nc.vector.tensor_copy(wT_sb[:], ps_wT[:])
                nc.sync.dma_start(
                    attn_out[b].rearrange("(tt p) -> tt p", p=128), wT_sb[:]
                )

    nc.compile()
    return nc


def _get_nc():
    if "nc" not in _CACHE:
        _CACHE["nc"] = _build()
    return _CACHE["nc"]


def kernel(encoder_features, decoder_hidden, W_enc, b_enc, W_dec, v_att, b_att):
    from concourse.bass_utils import run_bass_kernel_spmd

    nc = _get_nc()
    encoder_features = np.ascontiguousarray(encoder_features, dtype=np.float32)
    decoder_hidden = np.ascontiguousarray(decoder_hidden, dtype=np.float32)
    shared = {
        "w_enc": np.ascontiguousarray(W_enc, dtype=np.float32),
        "b_enc": np.ascontiguousarray(b_enc, dtype=np.float32),
        "w_dec": np.ascontiguousarray(W_dec, dtype=np.float32),
        "v_att": np.ascontiguousarray(v_att, dtype=np.float32),
    }
    in_maps = []
    for c in range(N_CORES):
        sl = slice(c * BPC, (c + 1) * BPC)
        in_maps.append(
            dict(
                enc=np.ascontiguousarray(encoder_features[sl]),
                dec=np.ascontiguousarray(decoder_hidden[sl]),
                **shared,
            )
        )
    trace = bool(int(os.environ.get("KERNEL_TRACE", "0")))
    res = run_bass_kernel_spmd(
        nc, in_maps, core_ids=list(range(N_CORES)), trace=trace
    )
    if trace:
        _CACHE["last_result"] = res
    ctx = np.concatenate([res.results[c]["ctx_out"] for c in range(N_CORES)], axis=0)
    attn = np.concatenate([res.results[c]["attn_out"] for c in range(N_CORES)], axis=0)
    return ctx, attn


# revision 21
# speedup vs baseline: 1.0207x; 1.0207x over previous
"""Bahdanau-style attention on 8 Trainium2 NeuronCores.

Per batch row b (shard: 8 batch rows per core, weights replicated):
    proj   = enc[b] @ W_enc + b_enc            # (T, A)
    energy = tanh(proj + dec[b] @ W_dec)       # (T, A)
    scores = energy @ v_att                    # (T,)  (+b_att cancels in softmax)
    w      = softmax(scores)
    out    = (w @ enc[b], w)

Core dataflow per batch:
  1. SWDGE cast-DMA loads enc fp32 -> bf16 native (t-parts, e-free); the
     dtype cast is free (runs at full HBM rate).
  2. encT (e-parts, t-free) built two ways in parallel (they use different
     resources): XBAR dma-transpose for the first XBAR_TT t-blocks (shares
     SDMA capacity with the loads), PE matmul-vs-identity transposes +
     DVE/ACT psum drains for the rest.
  3. projT chunks = W.T @ encT on PE (W stationary, fp32 accum), tanh+bias
     applied by ACT straight out of PSUM (bias = dec_projT + b_enc,
     per-partition AP), energy stored bf16 (a-parts, t-free).
  4. scores = v.T @ energy on PE (v stationary, streams energy, N=512
     chunks) -> (1, T) psum rows; softmax on the single lane: ACT exp with
     free-dim accumulate, DVE reciprocal + scale. No max-subtraction:
     |scores| <= sum|v| < 16, exp is safe in fp32.
  5. w row (bf16) roundtrips DRAM and comes back XBAR-transposed as
     (128 t-parts, 32) for the context matmul.
  6. context = w.T @ enc_bf on PE (w column stationary, streams native enc).
"""

import os
import sys

import numpy as np

for _p in ("/opt/trn_rl_repo",):
    if _p not in sys.path:
        sys.path.insert(0, _p)

B, T, E, A = 64, 4096, 512, 256
N_CORES = 8
BPC = B // N_CORES          # batches per core
TT = T // 128               # 32 t-blocks
EJ = E // 128               # 4 e-blocks
AM = A // 128               # 2 a-halves
TC = 512                    # t-chunk (psum bank)
NTC = T // TC               # 8 chunks
XBAR_TT = 12                # t-blocks transposed via XBAR DMA (rest: PE)

_CACHE = {}


def _build():
    import concourse.mybir as mybir
    import concourse.tile as tile
    from concourse import bacc
    from concourse.masks import make_identity

    fp32 = mybir.dt.float32
    bf16 = mybir.dt.bfloat16

    nc = bacc.Bacc("TRN2", target_bir_lowering=False, num_devices=N_CORES)

    enc = nc.dram_tensor("enc", [BPC, T, E], fp32, kind="ExternalInput").ap()
    dec = nc.dram_tensor("dec", [BPC, E], fp32, kind="ExternalInput").ap()
    w_enc = nc.dram_tensor("w_enc", [E, A], fp32, kind="ExternalInput").ap()
    b_enc = nc.dram_tensor("b_enc", [A], fp32, kind="ExternalInput").ap()
    w_dec = nc.dram_tensor("w_dec", [E, A], fp32, kind="ExternalInput").ap()
    v_att = nc.dram_tensor("v_att", [A], fp32, kind="ExternalInput").ap()
    ctx_out = nc.dram_tensor("ctx_out", [BPC, E], fp32, kind="ExternalOutput").ap()
    attn_out = nc.dram_tensor("attn_out", [BPC, T], fp32, kind="ExternalOutput").ap()

    with tile.TileContext(nc) as tc:
        with (
            tc.tile_pool(name="const", bufs=1) as cpool,
            tc.tile_pool(name="enc", bufs=3) as enc_pool,
            tc.tile_pool(name="encT", bufs=1) as encT_pool,
            tc.tile_pool(name="energy", bufs=2) as en_pool,
            tc.tile_pool(name="small", bufs=2) as spool,
            tc.tile_pool(name="psum_proj", bufs=2, space="PSUM") as pp,
            tc.tile_pool(name="psum_tr", bufs=2, space="PSUM") as pt,
            tc.tile_pool(name="psum_sc", bufs=2, space="PSUM") as psc,
            tc.tile_pool(name="psum_ctx", bufs=1, space="PSUM") as pctx,
            tc.tile_pool(name="psum_misc", bufs=1, space="PSUM") as pmisc,
        ):
            # ---- identities + first enc load go first: the SWDGE queue is
            # FIFO and the first batch's transposes gate the whole pipeline.
            ident_bf = cpool.tile([128, 128], bf16)
            make_identity(nc, ident_bf[:])
            ident_f = cpool.tile([128, 128], fp32)
            make_identity(nc, ident_f[:])
            enc_bf0 = enc_pool.tile([128, TT, E], bf16, tag="enc_bf")
            nc.gpsimd.dma_start(
                enc_bf0[:], enc[0].rearrange("(tt p) e -> p tt e", p=128)
            )

            # ---- constants / one-time prep -------------------------------
            wenc_bf = cpool.tile([128, EJ, A], bf16)        # [e_in, e_blk, a]
            nc.gpsimd.dma_start(
                wenc_bf[:], w_enc.rearrange("(j p) a -> p j a", p=128)
            )
            wdec_bf = cpool.tile([128, EJ, A], bf16)
            nc.gpsimd.dma_start(
                wdec_bf[:], w_dec.rearrange("(j p) a -> p j a", p=128)
            )
            v_sb = cpool.tile([128, AM], bf16)              # v[m*128+p]
            b_sb = cpool.tile([128, AM], fp32)
            for m in range(AM):
                nc.gpsimd.dma_start(v_sb[:, m : m + 1], v_att[m * 128 : (m + 1) * 128, None])
                nc.sync.dma_start(b_sb[:, m : m + 1], b_enc[m * 128 : (m + 1) * 128, None])

            ones_col = cpool.tile([128, 1], fp32)
            nc.vector.memset(ones_col[:], 1.0)
            ones_row = cpool.tile([1, 128], fp32)
            nc.vector.memset(ones_row[:], 1.0)

            # decoder projection -> per-batch per-partition bias
            dec_bf = cpool.tile([8, E], bf16)
            nc.gpsimd.dma_start(dec_bf[:], dec[:, :])
            decT_bf = cpool.tile([128, EJ, BPC], bf16)      # dec.T blocks
            for j in range(EJ):
                ps_t = pmisc.tile([128, BPC], bf16, tag="misc")
                nc.tensor.transpose(
                    ps_t[:], dec_bf[:, j * 128 : (j + 1) * 128], ident_bf[:8, :8]
                )
                nc.vector.tensor_copy(decT_bf[:, j, :], ps_t[:])
            biasT = cpool.tile([128, AM, BPC], fp32)        # dprojT + b_enc
            for m in range(AM):
                ps_dp = pmisc.tile([128, BPC], fp32, tag="misc")
                for j in range(EJ):
                    nc.tensor.matmul(
                        ps_dp[:],
                        lhsT=wdec_bf[:, j, m * 128 : (m + 1) * 128],
                        rhs=decT_bf[:, j, :],
                        start=(j == 0),
                        stop=(j == EJ - 1),
                    )
                nc.vector.tensor_tensor(
                    biasT[:, m, :],
                    ps_dp[:],
                    b_sb[:, m : m + 1].to_broadcast((128, BPC)),
                    mybir.AluOpType.add,
                )

            # ---- main per-batch pipeline ---------------------------------
            for b in range(BPC):
                # 1. cast-load native: enc_bf[p, tt, e] = enc[tt*128+p, e]
                if b == 0:
                    enc_bf = enc_bf0
                else:
                    enc_bf = enc_pool.tile([128, TT, E], bf16, tag="enc_bf")
                    nc.gpsimd.dma_start(
                        enc_bf[:], enc[b].rearrange("(tt p) e -> p tt e", p=128)
                    )
                # 2. encT2[p, tt, j*128+ti] = enc[tt*128+ti, j*128+p]
                encT2 = encT_pool.tile([128, TT, EJ * 128], bf16, tag="encT")
                # 2a. XBAR for the last XBAR_TT t-blocks, groups of 4
                for tt0 in range(TT - XBAR_TT, TT, 4):
                    nc.sync.dma_start_transpose(
                        encT2[:, tt0 : tt0 + 4, :].rearrange(
                            "p tt (j ti) -> p (tt j) ti", ti=128
                        ),
                        enc_bf[:, tt0 : tt0 + 4, :].rearrange("p tt e -> p (tt e)"),
                    )
                # 2b. PE transposes for the first blocks; drains on DVE
                for tt in range(0, TT - XBAR_TT):
                    ps_tr = pt.tile([128, 512], fp32, tag="tr")
                    for k in range(EJ):
                        nc.tensor.matmul(
                            ps_tr[:, k * 128 : (k + 1) * 128],
                            lhsT=enc_bf[:, tt, k * 128 : (k + 1) * 128],
                            rhs=ident_bf[:],
                        )
                    nc.vector.tensor_copy(encT2[:, tt, :], ps_tr[:])
                # 3. projection chunks + tanh
                energy = en_pool.tile([128, AM, T], bf16, tag="energy")
                for tcx in range(NTC):
                    for m in range(AM):
                        ps = pp.tile([128, TC], fp32, tag="proj")
                        for j in range(EJ):
                            nc.tensor.matmul(
                                ps[:],
                                lhsT=wenc_bf[:, j, m * 128 : (m + 1) * 128],
                                rhs=encT2[:, 4 * tcx : 4 * tcx + 4, j * 128 : (j + 1) * 128],
                                start=(j == 0),
                                stop=(j == EJ - 1),
                            )
                        nc.scalar.activation(
                            energy[:, m, tcx * TC : (tcx + 1) * TC],
                            ps[:],
                            mybir.ActivationFunctionType.Tanh,
                            bias=biasT[:, m, b : b + 1],
                            scale=1.0,
                        )

                # 4. scores in (128 t-parts, 32)
                ps_sc = psc.tile([128, TT], fp32, tag="scores")
                for tt in range(TT):
                    for m in range(AM):
                        nc.tensor.matmul(
                            ps_sc[:, tt : tt + 1],
                            lhsT=energy[:, m, tt * 128 : (tt + 1) * 128],
                            rhs=v_sb[:, m : m + 1],
                            start=(m == 0),
                            stop=(m == AM - 1),
                        )
                # 5. softmax
                expw = spool.tile([128, TT], fp32, tag="expw")
                sumrow = spool.tile([128, 1], fp32, tag="sumrow")
                nc.scalar.activation(
                    expw[:],
                    ps_sc[:],
                    mybir.ActivationFunctionType.Exp,
                    accum_out=sumrow[:],
                )
                ps_tot = pmisc.tile([1, 1], fp32, tag="misc")
                nc.tensor.matmul(ps_tot[:], lhsT=ones_col[:], rhs=sumrow[:])
                inv = spool.tile([1, 1], fp32, tag="inv")
                nc.vector.reciprocal(inv[:], ps_tot[:])
                ps_invb = pmisc.tile([128, 1], fp32, tag="misc")
                nc.tensor.matmul(ps_invb[:], lhsT=ones_row[:], rhs=inv[:])
                w_f = spool.tile([128, TT], fp32, tag="w_f")
                nc.vector.tensor_tensor(
                    w_f[:],
                    expw[:],
                    ps_invb[:].to_broadcast((128, TT)),
                    mybir.AluOpType.mult,
                )
                w_bf = spool.tile([128, TT], bf16, tag="w_bf")
                nc.vector.tensor_copy(w_bf[:], w_f[:])
                # 6. context
                ps_cx = pctx.tile([1, E], fp32, tag="ctx")
                for tt in range(TT):
                    nc.tensor.matmul(
                        ps_cx[:],
                        lhsT=w_bf[:, tt : tt + 1],
                        rhs=enc_bf[:, tt, :],
                        start=(tt == 0),
                        stop=(tt == TT - 1),
                    )
                cx_sb = spool.tile([1, E], fp32, tag="cx_sb")
                nc.vector.tensor_copy(cx_sb[:], ps_cx[:])
                nc.sync.dma_start(ctx_out[b, None, :], cx_sb[:])
                # 7. weights out
                ps_wT = pmisc.tile([32, 128], fp32, tag="misc")
                nc.tensor.transpose(ps_wT[:], w_f[:], ident_f[:])
                wT_sb = spool.tile([32, 128], fp32, tag="wT_sb")
                nc.vector.tensor_copy(wT_sb[:], ps_wT[:])
                nc.sync.dma_start(
                    attn_out[b].rearrange("(tt p) -> tt p", p=128), wT_sb[:]
                )

    nc.compile()
    return nc


def _get_nc():
    if "nc" not in _CACHE:
        _CACHE["nc"] = _build()
    return _CACHE["nc"]


def kernel(encoder_features, decoder_hidden, W_enc, b_enc, W_dec, v_att, b_att):
    from concourse.bass_utils import run_bass_kernel_spmd

    nc = _get_nc()
    encoder_features = np.ascontiguousarray(encoder_features, dtype=np.float32)
    decoder_hidden = np.ascontiguousarray(decoder_hidden, dtype=np.float32)
    shared = {
        "w_enc": np.ascontiguousarray(W_enc, dtype=np.float32),
        "b_enc": np.ascontiguousarray(b_enc, dtype=np.float32),
        "w_dec": np.ascontiguousarray(W_dec, dtype=np.float32),
        "v_att": np.ascontiguousarray(v_att, dtype=np.float32),
    }
    in_maps = []
    for c in range(N_CORES):
        sl = slice(c * BPC, (c + 1) * BPC)
        in_maps.append(
            dict(
                enc=np.ascontiguousarray(encoder_features[sl]),
                dec=np.ascontiguousarray(decoder_hidden[sl]),
                **shared,
            )
        )
    trace = bool(int(os.environ.get("KERNEL_TRACE", "0")))
    res = run_bass_kernel_spmd(
        nc, in_maps, core_ids=list(range(N_CORES)), trace=trace
    )
    if trace:
        _CACHE["last_result"] = res
    ctx = np.concatenate([res.results[c]["ctx_out"] for c in range(N_CORES)], axis=0)
    attn = np.concatenate([res.results[c]["attn_out"] for c in range(N_CORES)], axis=0)
    return ctx, attn


# revision 27
# speedup vs baseline: 1.0955x; 1.0733x over previous
"""Bahdanau-style attention on 8 Trainium2 NeuronCores.

Per batch row b (shard: 8 batch rows per core, weights replicated):
    proj   = enc[b] @ W_enc + b_enc            # (T, A)
    energy = tanh(proj + dec[b] @ W_dec)       # (T, A)
    scores = energy @ v_att                    # (T,)  (+b_att cancels in softmax)
    w      = softmax(scores)
    out    = (w @ enc[b], w)

Core dataflow per batch:
  1. SWDGE cast-DMA loads enc fp32 -> bf16 native (t-parts, e-free); the
     dtype cast is free (runs at full HBM rate).
  2. encT (e-parts, t-free) built two ways in parallel (they use different
     resources): XBAR dma-transpose for the first XBAR_TT t-blocks (shares
     SDMA capacity with the loads), PE matmul-vs-identity transposes +
     DVE/ACT psum drains for the rest.
  3. projT chunks = W.T @ encT on PE (W stationary, fp32 accum), tanh+bias
     applied by ACT straight out of PSUM (bias = dec_projT + b_enc,
     per-partition AP), energy stored bf16 (a-parts, t-free).
  4. scores = v.T @ energy on PE (v stationary, streams energy, N=512
     chunks) -> (1, T) psum rows; softmax on the single lane: ACT exp with
     free-dim accumulate, DVE reciprocal + scale. No max-subtraction:
     |scores| <= sum|v| < 16, exp is safe in fp32.
  5. w row (bf16) roundtrips DRAM and comes back XBAR-transposed as
     (128 t-parts, 32) for the context matmul.
  6. context = w.T @ enc_bf on PE (w column stationary, streams native enc).
"""

import os
import sys

import numpy as np

for _p in ("/opt/trn_rl_repo",):
    if _p not in sys.path:
        sys.path.insert(0, _p)

B, T, E, A = 64, 4096, 512, 256
N_CORES = 8
BPC = B // N_CORES          # batches per core
TT = T // 128               # 32 t-blocks
EJ = E // 128               # 4 e-blocks
AM = A // 128               # 2 a-halves
TC = 512                    # t-chunk (psum bank)
NTC = T // TC               # 8 chunks
XBAR_TT = 8                # t-blocks transposed via XBAR DMA (rest: PE)

_CACHE = {}


def _build():
    import concourse.mybir as mybir
    import concourse.tile as tile
    from concourse import bacc
    from concourse.masks import make_identity

    fp32 = mybir.dt.float32
    bf16 = mybir.dt.bfloat16

    nc = bacc.Bacc("TRN2", target_bir_lowering=False, num_devices=N_CORES)

    enc = nc.dram_tensor("enc", [BPC, T, E], fp32, kind="ExternalInput").ap()
    dec = nc.dram_tensor("dec", [BPC, E], fp32, kind="ExternalInput").ap()
    w_enc = nc.dram_tensor("w_enc", [E, A], fp32, kind="ExternalInput").ap()
    b_enc = nc.dram_tensor("b_enc", [A], fp32, kind="ExternalInput").ap()
    w_dec = nc.dram_tensor("w_dec", [E, A], fp32, kind="ExternalInput").ap()
    v_att = nc.dram_tensor("v_att", [A], fp32, kind="ExternalInput").ap()
    ctx_out = nc.dram_tensor("ctx_out", [BPC, E], fp32, kind="ExternalOutput").ap()
    attn_out = nc.dram_tensor("attn_out", [BPC, T], fp32, kind="ExternalOutput").ap()

    with tile.TileContext(nc) as tc:
        with (
            tc.tile_pool(name="const", bufs=1) as cpool,
            tc.tile_pool(name="enc", bufs=3) as enc_pool,
            tc.tile_pool(name="encT", bufs=1) as encT_pool,
            tc.tile_pool(name="energy", bufs=2) as en_pool,
            tc.tile_pool(name="small", bufs=3) as spool,
            tc.tile_pool(name="psum_proj", bufs=2, space="PSUM") as pp,
            tc.tile_pool(name="psum_tr", bufs=2, space="PSUM") as pt,
            tc.tile_pool(name="psum_sc", bufs=2, space="PSUM") as psc,
            tc.tile_pool(name="psum_ctx", bufs=1, space="PSUM") as pctx,
            tc.tile_pool(name="psum_misc", bufs=1, space="PSUM") as pmisc,
        ):
            # ---- identities + first enc load go first: the SWDGE queue is
            # FIFO and the first batch's transposes gate the whole pipeline.
            ident_bf = cpool.tile([128, 128], bf16)
            make_identity(nc, ident_bf[:])
            ident_f = cpool.tile([128, 128], fp32)
            make_identity(nc, ident_f[:])
            enc_bf0 = enc_pool.tile([128, TT, E], bf16, tag="enc_bf")
            nc.gpsimd.dma_start(
                enc_bf0[:], enc[0].rearrange("(tt p) e -> p tt e", p=128)
            )

            # ---- constants / one-time prep -------------------------------
            wenc_bf = cpool.tile([128, EJ, A], bf16)        # [e_in, e_blk, a]
            nc.gpsimd.dma_start(
                wenc_bf[:], w_enc.rearrange("(j p) a -> p j a", p=128)
            )
            wdec_bf = cpool.tile([128, EJ, A], bf16)
            nc.gpsimd.dma_start(
                wdec_bf[:], w_dec.rearrange("(j p) a -> p j a", p=128)
            )
            v_sb = cpool.tile([128, AM], bf16)              # v[m*128+p]
            b_sb = cpool.tile([128, AM], fp32)
            for m in range(AM):
                nc.gpsimd.dma_start(v_sb[:, m : m + 1], v_att[m * 128 : (m + 1) * 128, None])
                nc.sync.dma_start(b_sb[:, m : m + 1], b_enc[m * 128 : (m + 1) * 128, None])

            ones_col = cpool.tile([128, 1], fp32)
            nc.vector.memset(ones_col[:], 1.0)
            ones_row = cpool.tile([1, 128], fp32)
            nc.vector.memset(ones_row[:], 1.0)

            # decoder projection -> per-batch per-partition bias
            dec_bf = cpool.tile([8, E], bf16)
            nc.gpsimd.dma_start(dec_bf[:], dec[:, :])
            decT_bf = cpool.tile([128, EJ, BPC], bf16)      # dec.T blocks
            for j in range(EJ):
                ps_t = pmisc.tile([128, BPC], bf16, tag="misc")
                nc.tensor.transpose(
                    ps_t[:], dec_bf[:, j * 128 : (j + 1) * 128], ident_bf[:8, :8]
                )
                nc.vector.tensor_copy(decT_bf[:, j, :], ps_t[:])
            biasT = cpool.tile([128, AM, BPC], fp32)        # dprojT + b_enc
            for m in range(AM):
                ps_dp = pmisc.tile([128, BPC], fp32, tag="misc")
                for j in range(EJ):
                    nc.tensor.matmul(
                        ps_dp[:],
                        lhsT=wdec_bf[:, j, m * 128 : (m + 1) * 128],
                        rhs=decT_bf[:, j, :],
                        start=(j == 0),
                        stop=(j == EJ - 1),
                    )
                nc.vector.tensor_tensor(
                    biasT[:, m, :],
                    ps_dp[:],
                    b_sb[:, m : m + 1].to_broadcast((128, BPC)),
                    mybir.AluOpType.add,
                )

            # ---- main per-batch pipeline ---------------------------------
            for b in range(BPC):
                # 1. cast-load native: enc_bf[p, tt, e] = enc[tt*128+p, e]
                if b == 0:
                    enc_bf = enc_bf0
                else:
                    enc_bf = enc_pool.tile([128, TT, E], bf16, tag="enc_bf")
                    nc.gpsimd.dma_start(
                        enc_bf[:], enc[b].rearrange("(tt p) e -> p tt e", p=128)
                    )
                # 2. encT2[p, tt, j*128+ti] = enc[tt*128+ti, j*128+p]
                encT2 = encT_pool.tile([128, TT, EJ * 128], bf16, tag="encT")
                # 2a. XBAR for the last XBAR_TT t-blocks, groups of 4
                for tt0 in range(TT - XBAR_TT, TT, 4):
                    nc.sync.dma_start_transpose(
                        encT2[:, tt0 : tt0 + 4, :].rearrange(
                            "p tt (j ti) -> p (tt j) ti", ti=128
                        ),
                        enc_bf[:, tt0 : tt0 + 4, :].rearrange("p tt e -> p (tt e)"),
                    )
                # 2b. PE transposes for the first blocks; drains on DVE
                for tt in range(0, TT - XBAR_TT):
                    ps_tr = pt.tile([128, 512], fp32, tag="tr")
                    for k in range(EJ):
                        nc.tensor.matmul(
                            ps_tr[:, k * 128 : (k + 1) * 128],
                            lhsT=enc_bf[:, tt, k * 128 : (k + 1) * 128],
                            rhs=ident_bf[:],
                        )
                    nc.vector.tensor_copy(encT2[:, tt, :], ps_tr[:])
                # 3. projection chunks + tanh
                energy = en_pool.tile([128, AM, T], bf16, tag="energy")
                for tcx in range(NTC):
                    for m in range(AM):
                        ps = pp.tile([128, TC], fp32, tag="proj")
                        for j in range(EJ):
                            nc.tensor.matmul(
                                ps[:],
                                lhsT=wenc_bf[:, j, m * 128 : (m + 1) * 128],
                                rhs=encT2[:, 4 * tcx : 4 * tcx + 4, j * 128 : (j + 1) * 128],
                                start=(j == 0),
                                stop=(j == EJ - 1),
                            )
                        nc.scalar.activation(
                            energy[:, m, tcx * TC : (tcx + 1) * TC],
                            ps[:],
                            mybir.ActivationFunctionType.Tanh,
                            bias=biasT[:, m, b : b + 1],
                            scale=1.0,
                        )

                # 4. scores in (128 t-parts, 32)
                ps_sc = psc.tile([128, TT], fp32, tag="scores")
                for tt in range(TT):
                    for m in range(AM):
                        nc.tensor.matmul(
                            ps_sc[:, tt : tt + 1],
                            lhsT=energy[:, m, tt * 128 : (tt + 1) * 128],
                            rhs=v_sb[:, m : m + 1],
                            start=(m == 0),
                            stop=(m == AM - 1),
                        )
                # 5. softmax
                expw = spool.tile([128, TT], fp32, tag="expw")
                sumrow = spool.tile([128, 1], fp32, tag="sumrow")
                nc.scalar.activation(
                    expw[:],
                    ps_sc[:],
                    mybir.ActivationFunctionType.Exp,
                    accum_out=sumrow[:],
                )
                ps_tot = pmisc.tile([1, 1], fp32, tag="misc")
                nc.tensor.matmul(ps_tot[:], lhsT=ones_col[:], rhs=sumrow[:])
                inv = spool.tile([1, 1], fp32, tag="inv")
                nc.vector.reciprocal(inv[:], ps_tot[:])
                ps_invb = pmisc.tile([128, 1], fp32, tag="misc")
                nc.tensor.matmul(ps_invb[:], lhsT=ones_row[:], rhs=inv[:])
                w_f = spool.tile([128, TT], fp32, tag="w_f")
                nc.vector.tensor_tensor(
                    w_f[:],
                    expw[:],
                    ps_invb[:].to_broadcast((128, TT)),
                    mybir.AluOpType.mult,
                )
                w_bf = spool.tile([128, TT], bf16, tag="w_bf")
                nc.vector.tensor_copy(w_bf[:], w_f[:])
                # 6. context
                ps_cx = pctx.tile([1, E], fp32, tag="ctx")
                for tt in range(TT):
                    nc.tensor.matmul(
                        ps_cx[:],
                        lhsT=w_bf[:, tt : tt + 1],
                        rhs=enc_bf[:, tt, :],
                        start=(tt == 0),
                        stop=(tt == TT - 1),
                    )
                cx_sb = spool.tile([1, E], fp32, tag="cx_sb")
                nc.vector.tensor_copy(cx_sb[:], ps_cx[:])
                nc.sync.dma_start(ctx_out[b, None, :], cx_sb[:])
                # 7. weights out
                ps_wT = pmisc.tile([32, 128], fp32, tag="misc")
                nc.tensor.transpose(ps_wT[:], w_f[:], ident_f[:])
                wT_sb = spool.tile([32, 128], fp32, tag="wT_sb")
                nc.vector.tensor_copy(wT_sb[:], ps_wT[:])
                nc.sync.dma_start(
                    attn_out[b].rearrange("(tt p) -> tt p", p=128), wT_sb[:]
                )

    nc.compile()
    return nc


def _get_nc():
    if "nc" not in _CACHE:
        _CACHE["nc"] = _build()
    return _CACHE["nc"]


def kernel(encoder_features, decoder_hidden, W_enc, b_enc, W_dec, v_att, b_att):
    from concourse.bass_utils import run_bass_kernel_spmd

    nc = _get_nc()
    encoder_features = np.ascontiguousarray(encoder_features, dtype=np.float32)
    decoder_hidden = np.ascontiguousarray(decoder_hidden, dtype=np.float32)
    shared = {
        "w_enc": np.ascontiguousarray(W_enc, dtype=np.float32),
        "b_enc": np.ascontiguousarray(b_enc, dtype=np.float32),
        "w_dec": np.ascontiguousarray(W_dec, dtype=np.float32),
        "v_att": np.ascontiguousarray(v_att, dtype=np.float32),
    }
    in_maps = []
    for c in range(N_CORES):
        sl = slice(c * BPC, (c + 1) * BPC)
        in_maps.append(
            dict(
                enc=np.ascontiguousarray(encoder_features[sl]),
                dec=np.ascontiguousarray(decoder_hidden[sl]),
                **shared,
            )
        )
    trace = bool(int(os.environ.get("KERNEL_TRACE", "0")))
    res = run_bass_kernel_spmd(
        nc, in_maps, core_ids=list(range(N_CORES)), trace=trace
    )
    if trace:
        _CACHE["last_result"] = res
    ctx = np.concatenate([res.results[c]["ctx_out"] for c in range(N_CORES)], axis=0)
    attn = np.concatenate([res.results[c]["attn_out"] for c in range(N_CORES)], axis=0)
    return ctx, attn


# revision 29
# speedup vs baseline: 1.1839x; 1.0807x over previous
"""Bahdanau-style attention on 8 Trainium2 NeuronCores.

Per batch row b (shard: 8 batch rows per core, weights replicated):
    proj   = enc[b] @ W_enc + b_enc            # (T, A)
    energy = tanh(proj + dec[b] @ W_dec)       # (T, A)
    scores = energy @ v_att                    # (T,)  (+b_att cancels in softmax)
    w      = softmax(scores)
    out    = (w @ enc[b], w)

Core dataflow per batch:
  1. SWDGE cast-DMA loads enc fp32 -> bf16 native (t-parts, e-free); the
     dtype cast is free (runs at full HBM rate).
  2. encT (e-parts, t-free) built two ways in parallel (they use different
     resources): XBAR dma-transpose for the first XBAR_TT t-blocks (shares
     SDMA capacity with the loads), PE matmul-vs-identity transposes +
     DVE/ACT psum drains for the rest.
  3. projT chunks = W.T @ encT on PE (W stationary, fp32 accum), tanh+bias
     applied by ACT straight out of PSUM (bias = dec_projT + b_enc,
     per-partition AP), energy stored bf16 (a-parts, t-free).
  4. scores = v.T @ energy on PE (v stationary, streams energy, N=512
     chunks) -> (1, T) psum rows; softmax on the single lane: ACT exp with
     free-dim accumulate, DVE reciprocal + scale. No max-subtraction:
     |scores| <= sum|v| < 16, exp is safe in fp32.
  5. w row (bf16) roundtrips DRAM and comes back XBAR-transposed as
     (128 t-parts, 32) for the context matmul.
  6. context = w.T @ enc_bf on PE (w column stationary, streams native enc).
"""

import os
import sys

import numpy as np

for _p in ("/opt/trn_rl_repo",):
    if _p not in sys.path:
        sys.path.insert(0, _p)

B, T, E, A = 64, 4096, 512, 256
N_CORES = 8
BPC = B // N_CORES          # batches per core
TT = T // 128               # 32 t-blocks
EJ = E // 128               # 4 e-blocks
AM = A // 128               # 2 a-halves
TC = 512                    # t-chunk (psum bank)
NTC = T // TC               # 8 chunks
XBAR_TT = 0                # t-blocks transposed via XBAR DMA (rest: PE)

_CACHE = {}


def _build():
    import concourse.mybir as mybir
    import concourse.tile as tile
    from concourse import bacc
    from concourse.masks import make_identity

    fp32 = mybir.dt.float32
    bf16 = mybir.dt.bfloat16

    nc = bacc.Bacc("TRN2", target_bir_lowering=False, num_devices=N_CORES)

    enc = nc.dram_tensor("enc", [BPC, T, E], fp32, kind="ExternalInput").ap()
    dec = nc.dram_tensor("dec", [BPC, E], fp32, kind="ExternalInput").ap()
    w_enc = nc.dram_tensor("w_enc", [E, A], fp32, kind="ExternalInput").ap()
    b_enc = nc.dram_tensor("b_enc", [A], fp32, kind="ExternalInput").ap()
    w_dec = nc.dram_tensor("w_dec", [E, A], fp32, kind="ExternalInput").ap()
    v_att = nc.dram_tensor("v_att", [A], fp32, kind="ExternalInput").ap()
    ctx_out = nc.dram_tensor("ctx_out", [BPC, E], fp32, kind="ExternalOutput").ap()
    attn_out = nc.dram_tensor("attn_out", [BPC, T], fp32, kind="ExternalOutput").ap()

    with tile.TileContext(nc) as tc:
        with (
            tc.tile_pool(name="const", bufs=1) as cpool,
            tc.tile_pool(name="enc", bufs=3) as enc_pool,
            tc.tile_pool(name="encT", bufs=1) as encT_pool,
            tc.tile_pool(name="energy", bufs=2) as en_pool,
            tc.tile_pool(name="small", bufs=3) as spool,
            tc.tile_pool(name="psum_proj", bufs=2, space="PSUM") as pp,
            tc.tile_pool(name="psum_tr", bufs=2, space="PSUM") as pt,
            tc.tile_pool(name="psum_sc", bufs=2, space="PSUM") as psc,
            tc.tile_pool(name="psum_ctx", bufs=1, space="PSUM") as pctx,
            tc.tile_pool(name="psum_misc", bufs=1, space="PSUM") as pmisc,
        ):
            # ---- identities + first enc load go first: the SWDGE queue is
            # FIFO and the first batch's transposes gate the whole pipeline.
            ident_bf = cpool.tile([128, 128], bf16)
            make_identity(nc, ident_bf[:])
            ident_f = cpool.tile([128, 128], fp32)
            make_identity(nc, ident_f[:])
            enc_bf0 = enc_pool.tile([128, TT, E], bf16, tag="enc_bf")
            nc.gpsimd.dma_start(
                enc_bf0[:], enc[0].rearrange("(tt p) e -> p tt e", p=128)
            )

            # ---- constants / one-time prep -------------------------------
            wenc_bf = cpool.tile([128, EJ, A], bf16)        # [e_in, e_blk, a]
            nc.gpsimd.dma_start(
                wenc_bf[:], w_enc.rearrange("(j p) a -> p j a", p=128)
            )
            wdec_bf = cpool.tile([128, EJ, A], bf16)
            nc.gpsimd.dma_start(
                wdec_bf[:], w_dec.rearrange("(j p) a -> p j a", p=128)
            )
            v_sb = cpool.tile([128, AM], bf16)              # v[m*128+p]
            b_sb = cpool.tile([128, AM], fp32)
            for m in range(AM):
                nc.gpsimd.dma_start(v_sb[:, m : m + 1], v_att[m * 128 : (m + 1) * 128, None])
                nc.sync.dma_start(b_sb[:, m : m + 1], b_enc[m * 128 : (m + 1) * 128, None])

            ones_col = cpool.tile([128, 1], fp32)
            nc.vector.memset(ones_col[:], 1.0)
            ones_row = cpool.tile([1, 128], fp32)
            nc.vector.memset(ones_row[:], 1.0)

            # decoder projection -> per-batch per-partition bias
            dec_bf = cpool.tile([8, E], bf16)
            nc.gpsimd.dma_start(dec_bf[:], dec[:, :])
            decT_bf = cpool.tile([128, EJ, BPC], bf16)      # dec.T blocks
            for j in range(EJ):
                ps_t = pmisc.tile([128, BPC], bf16, tag="misc")
                nc.tensor.transpose(
                    ps_t[:], dec_bf[:, j * 128 : (j + 1) * 128], ident_bf[:8, :8]
                )
                nc.vector.tensor_copy(decT_bf[:, j, :], ps_t[:])
            biasT = cpool.tile([128, AM, BPC], fp32)        # dprojT + b_enc
            for m in range(AM):
                ps_dp = pmisc.tile([128, BPC], fp32, tag="misc")
                for j in range(EJ):
                    nc.tensor.matmul(
                        ps_dp[:],
                        lhsT=wdec_bf[:, j, m * 128 : (m + 1) * 128],
                        rhs=decT_bf[:, j, :],
                        start=(j == 0),
                        stop=(j == EJ - 1),
                    )
                nc.vector.tensor_tensor(
                    biasT[:, m, :],
                    ps_dp[:],
                    b_sb[:, m : m + 1].to_broadcast((128, BPC)),
                    mybir.AluOpType.add,
                )

            # ---- main per-batch pipeline ---------------------------------
            for b in range(BPC):
                # 1. cast-load native: enc_bf[p, tt, e] = enc[tt*128+p, e]
                if b == 0:
                    enc_bf = enc_bf0
                else:
                    enc_bf = enc_pool.tile([128, TT, E], bf16, tag="enc_bf")
                    nc.gpsimd.dma_start(
                        enc_bf[:], enc[b].rearrange("(tt p) e -> p tt e", p=128)
                    )
                # 2. encT2[p, tt, j*128+ti] = enc[tt*128+ti, j*128+p]
                encT2 = encT_pool.tile([128, TT, EJ * 128], bf16, tag="encT")
                # 2a. XBAR for the last XBAR_TT t-blocks, groups of 4
                for tt0 in range(TT - XBAR_TT, TT, 4):
                    nc.sync.dma_start_transpose(
                        encT2[:, tt0 : tt0 + 4, :].rearrange(
                            "p tt (j ti) -> p (tt j) ti", ti=128
                        ),
                        enc_bf[:, tt0 : tt0 + 4, :].rearrange("p tt e -> p (tt e)"),
                    )
                # 2b. PE transposes for the first blocks; drains on DVE
                for tt in range(0, TT - XBAR_TT):
                    ps_tr = pt.tile([128, 512], fp32, tag="tr")
                    for k in range(EJ):
                        nc.tensor.matmul(
                            ps_tr[:, k * 128 : (k + 1) * 128],
                            lhsT=enc_bf[:, tt, k * 128 : (k + 1) * 128],
                            rhs=ident_bf[:],
                        )
                    nc.vector.tensor_copy(encT2[:, tt, :], ps_tr[:])
                # 3. projection chunks + tanh
                energy = en_pool.tile([128, AM, T], bf16, tag="energy")
                for tcx in range(NTC):
                    for m in range(AM):
                        ps = pp.tile([128, TC], fp32, tag="proj")
                        for j in range(EJ):
                            nc.tensor.matmul(
                                ps[:],
                                lhsT=wenc_bf[:, j, m * 128 : (m + 1) * 128],
                                rhs=encT2[:, 4 * tcx : 4 * tcx + 4, j * 128 : (j + 1) * 128],
                                start=(j == 0),
                                stop=(j == EJ - 1),
                            )
                        nc.scalar.activation(
                            energy[:, m, tcx * TC : (tcx + 1) * TC],
                            ps[:],
                            mybir.ActivationFunctionType.Tanh,
                            bias=biasT[:, m, b : b + 1],
                            scale=1.0,
                        )

                # 4. scores in (128 t-parts, 32)
                ps_sc = psc.tile([128, TT], fp32, tag="scores")
                for tt in range(TT):
                    for m in range(AM):
                        nc.tensor.matmul(
                            ps_sc[:, tt : tt + 1],
                            lhsT=energy[:, m, tt * 128 : (tt + 1) * 128],
                            rhs=v_sb[:, m : m + 1],
                            start=(m == 0),
                            stop=(m == AM - 1),
                        )
                # 5. softmax
                expw = spool.tile([128, TT], fp32, tag="expw")
                sumrow = spool.tile([128, 1], fp32, tag="sumrow")
                nc.scalar.activation(
                    expw[:],
                    ps_sc[:],
                    mybir.ActivationFunctionType.Exp,
                    accum_out=sumrow[:],
                )
                ps_tot = pmisc.tile([1, 1], fp32, tag="misc")
                nc.tensor.matmul(ps_tot[:], lhsT=ones_col[:], rhs=sumrow[:])
                inv = spool.tile([1, 1], fp32, tag="inv")
                nc.vector.reciprocal(inv[:], ps_tot[:])
                ps_invb = pmisc.tile([128, 1], fp32, tag="misc")
                nc.tensor.matmul(ps_invb[:], lhsT=ones_row[:], rhs=inv[:])
                w_f = spool.tile([128, TT], fp32, tag="w_f")
                nc.vector.tensor_tensor(
                    w_f[:],
                    expw[:],
                    ps_invb[:].to_broadcast((128, TT)),
                    mybir.AluOpType.mult,
                )
                w_bf = spool.tile([128, TT], bf16, tag="w_bf")
                nc.vector.tensor_copy(w_bf[:], w_f[:])
                # 6. context
                ps_cx = pctx.tile([1, E], fp32, tag="ctx")
                for tt in range(TT):
                    nc.tensor.matmul(
                        ps_cx[:],
                        lhsT=w_bf[:, tt : tt + 1],
                        rhs=enc_bf[:, tt, :],
                        start=(tt == 0),
                        stop=(tt == TT - 1),
                    )
                cx_sb = spool.tile([1, E], fp32, tag="cx_sb")
                nc.vector.tensor_copy(cx_sb[:], ps_cx[:])
                nc.sync.dma_start(ctx_out[b, None, :], cx_sb[:])
                # 7. weights out
                ps_wT = pmisc.tile([32, 128], fp32, tag="misc")
                nc.tensor.transpose(ps_wT[:], w_f[:], ident_f[:])
                wT_sb = spool.tile([32, 128], fp32, tag="wT_sb")
                nc.vector.tensor_copy(wT_sb[:], ps_wT[:])
                nc.sync.dma_start(
                    attn_out[b].rearrange("(tt p) -> tt p", p=128), wT_sb[:]
                )

    nc.compile()
    return nc


def _get_nc():
    if "nc" not in _CACHE:
        _CACHE["nc"] = _build()
    return _CACHE["nc"]


def kernel(encoder_features, decoder_hidden, W_enc, b_enc, W_dec, v_att, b_att):
    from concourse.bass_utils import run_bass_kernel_spmd

    nc = _get_nc()
    encoder_features = np.ascontiguousarray(encoder_features, dtype=np.float32)
    decoder_hidden = np.ascontiguousarray(decoder_hidden, dtype=np.float32)
    shared = {
        "w_enc": np.ascontiguousarray(W_enc, dtype=np.float32),
        "b_enc": np.ascontiguousarray(b_enc, dtype=np.float32),
        "w_dec": np.ascontiguousarray(W_dec, dtype=np.float32),
        "v_att": np.ascontiguousarray(v_att, dtype=np.float32),
    }
    in_maps = []
    for c in range(N_CORES):
        sl = slice(c * BPC, (c + 1) * BPC)
        in_maps.append(
            dict(
                enc=np.ascontiguousarray(encoder_features[sl]),
                dec=np.ascontiguousarray(decoder_hidden[sl]),
                **shared,
            )
        )
    trace = bool(int(os.environ.get("KERNEL_TRACE", "0")))
    res = run_bass_kernel_spmd(
        nc, in_maps, core_ids=list(range(N_CORES)), trace=trace
    )
    if trace:
        _CACHE["last_result"] = res
    ctx = np.concatenate([res.results[c]["ctx_out"] for c in range(N_CORES)], axis=0)
    attn = np.concatenate([res.results[c]["attn_out"] for c in range(N_CORES)], axis=0)
    return ctx, attn
